# revision 14
# baseline (speedup 1.0000x reference)
"""Trainium2 Bass kernel for ConvNext-style GNN message passing.

Math (matches reference):
    kernel = kernel_basis @ kernel_W.T            # [E, C] per-edge depthwise kernel
    x1     = segment_sum(x[src] * kernel, dst)    # [N, C]
    h      = LayerNorm(x1 + conv_bias) -> MLP(gelu) -> layer_scale
    out    = layer_scale * h + x

Strategy: sort edges by dst on the host and shard them by destination-node
range across the 8 cores (each core owns N/8 = 12500 output rows -> no
collectives). Per core, edges are laid out in 128-edge "tiles" grouped by
128-node output chunks.  On device, per tile:
  - per-edge kernels via PE matmul (kbT_tile[32,128].T @ Wt[32,64])
  - x[src] rows fetched with one big batched indirect DMA gather (int32 idx)
  - message = gather * kernel on DVE (bf16)
  - one-hot(dst_offset) built on DVE via is_equal vs an iota row
  - segment-sum via PE matmul: onehot.T @ message accumulated in PSUM
Each finished 128-node chunk then runs LayerNorm (batched per 8 chunks) and
the MLP + layer-scale + residual, and is written straight out.

All streams are bf16 (fp32 accumulation in PSUM); the residual path stays
fp32.  Output = x + 1e-6 * h, so bf16 error in h is ~1e-8 relative in out.
"""

import sys

sys.path.insert(0, "/opt/trn_rl_repo")

from contextlib import ExitStack

import numpy as np
from ml_dtypes import bfloat16

import concourse.bacc as bacc
import concourse.bass as bass
import concourse.tile as tile
from concourse import mybir
from concourse.bass_utils import run_bass_kernel_spmd
from concourse.masks import make_identity

P = 128
NCORES = 8
BT = 64  # tiles per gather batch
GRP = 8  # chunks per LayerNorm group
KSUB = 8  # kernel matmuls per PSUM bank flush

f32 = mybir.dt.float32
bf16 = mybir.dt.bfloat16
i32 = mybir.dt.int32
Alu = mybir.AluOpType
Act = mybir.ActivationFunctionType


def _plan(src, dst, kernel_basis, n_nodes, n_cores):
    """Host-side edge layout. Returns shared schedule + per-core arrays."""
    E, KDIM = kernel_basis.shape
    NS = n_nodes // n_cores
    KCH = (NS + P - 1) // P  # chunks per core
    src = src.astype(np.int64)
    dst = dst.astype(np.int64)
    core = dst // NS
    chunk = (dst % NS) // P
    key = (core * KCH + chunk) * n_nodes + src
    order = np.argsort(key, kind="stable")
    core_s = core[order]
    chunk_s = chunk[order]
    src_s = src[order]
    dst_s = dst[order]

    cc = core_s * KCH + chunk_s
    counts = np.bincount(cc, minlength=n_cores * KCH).reshape(n_cores, KCH)
    # tiles per chunk: shared across cores (program is SPMD-identical)
    T_c = np.maximum(1, -(-counts.max(axis=0) // P)).astype(np.int64)
    Ttot = int(T_c.sum())
    S = Ttot * P
    tile_start = np.concatenate([[0], np.cumsum(T_c)])

    starts = np.searchsorted(cc, np.arange(n_cores * KCH), side="left")
    rank = np.arange(E) - starts[cc]
    slot = tile_start[chunk_s] * P + rank  # slot within the core's layout

    src_slots = np.zeros((n_cores, S), np.int32)
    off_slots = np.full((n_cores, S), 200.0, np.float32)  # pad -> one-hot row of 0
    kb_slots = np.zeros((n_cores, S, KDIM), np.float32)
    src_slots[core_s, slot] = src_s.astype(np.int32)
    off_slots[core_s, slot] = ((dst_s % NS) % P).astype(np.float32)
    kb_slots[core_s, slot, :] = kernel_basis[order]

    tile_chunk = np.repeat(np.arange(KCH), T_c)
    return dict(
        NS=NS,
        KCH=KCH,
        T_c=T_c,
        Ttot=Ttot,
        S=S,
        tile_start=tile_start,
        tile_chunk=tile_chunk,
        src_slots=src_slots,
        off_slots=off_slots,
        kb_slots=kb_slots,
    )


def _build_nc(n_nodes, C, KDIM, H, NS, KCH, Ttot, tile_start, tile_chunk,
              dbg=False):
    """Build the SPMD Bass program (shared across cores; data differs)."""
    nc = bacc.Bacc("TRN2", target_bir_lowering=False, debug=False, num_devices=NCORES)
    S = Ttot * P
    H2 = H // P  # H partition-chunks (2 for H=256)
    assert H == H2 * P and C <= P and KDIM <= P
    if dbg:
        xg_d = nc.dram_tensor("xg_d", [P, Ttot * C], bf16, kind="ExternalOutput")
        kern_d = nc.dram_tensor("kern_d", [P, Ttot * C], bf16, kind="ExternalOutput")
        msg_d = nc.dram_tensor("msg_d", [P, Ttot * C], bf16, kind="ExternalOutput")
        a_d = nc.dram_tensor("a_d", [P, Ttot * P], bf16, kind="ExternalOutput")
        x1_d = nc.dram_tensor("x1_d", [P, KCH * C], f32, kind="ExternalOutput")

    xg = nc.dram_tensor("xg", [n_nodes, C], f32, kind="ExternalInput")
    xadj = nc.dram_tensor("xadj", [NS, C], f32, kind="ExternalInput")
    kbt = nc.dram_tensor("kbt", [KDIM, S], bf16, kind="ExternalInput")
    sidx = nc.dram_tensor("sidx", [P, Ttot], i32, kind="ExternalInput")
    doff = nc.dram_tensor("doff", [P, Ttot], bf16, kind="ExternalInput")
    wt = nc.dram_tensor("wt", [KDIM, C], bf16, kind="ExternalInput")
    w1t = nc.dram_tensor("w1t", [C, H], bf16, kind="ExternalInput")
    w2t = nc.dram_tensor("w2t", [P, H2 * C], bf16, kind="ExternalInput")
    b1t = nc.dram_tensor("b1t", [P, H2], f32, kind="ExternalInput")
    rows = nc.dram_tensor("rows", [P, 4 * C], f32, kind="ExternalInput")
    iota = nc.dram_tensor("iota", [P, P], bf16, kind="ExternalInput")
    y = nc.dram_tensor("y", [NS, C], f32, kind="ExternalOutput")

    with tile.TileContext(nc) as tc, ExitStack() as ctx:
        const = ctx.enter_context(tc.tile_pool(name="const", bufs=1))
        gpool = ctx.enter_context(tc.tile_pool(name="gath", bufs=2))
        kpool = ctx.enter_context(tc.tile_pool(name="kbts", bufs=2))
        mpool = ctx.enter_context(tc.tile_pool(name="msg", bufs=2))
        apool = ctx.enter_context(tc.tile_pool(name="onehot", bufs=2))
        x1pool = ctx.enter_context(tc.tile_pool(name="x1", bufs=2))
        tpool = ctx.enter_context(tc.tile_pool(name="tmp", bufs=2))
        opool = ctx.enter_context(tc.tile_pool(name="outp", bufs=3))
        kmm_ps = ctx.enter_context(tc.tile_pool(name="kmm", bufs=2, space="PSUM"))
        acc_ps = ctx.enter_context(tc.tile_pool(name="acc", bufs=3, space="PSUM"))
        epi_ps = ctx.enter_context(tc.tile_pool(name="epi", bufs=3, space="PSUM"))

        def load_const(t, shape, dtype):
            sb = const.tile(shape, dtype, name=f"c_{t.name}", tag=f"c_{t.name}")
            nc.sync.dma_start(sb[:], t.ap())
            return sb

        doff_sb = load_const(doff, [P, Ttot], bf16)
        wt_sb = load_const(wt, [KDIM, C], bf16)
        w1t_sb = load_const(w1t, [C, H], bf16)
        w2t_sb = load_const(w2t, [P, H2 * C], bf16)
        b1t_sb = load_const(b1t, [P, H2], f32)
        rows_sb = load_const(rows, [P, 4 * C], f32)
        iota_sb = load_const(iota, [P, P], bf16)
        ident_sb = const.tile([P, P], f32)
        make_identity(nc, ident_sb[:])
        eps_sb = const.tile([P, 1], f32)
        nc.gpsimd.memset(eps_sb[:], 1e-5)

        cb_row = rows_sb[:, 0 * C : 1 * C]
        ga_row = rows_sb[:, 1 * C : 2 * C]
        be_row = rows_sb[:, 2 * C : 3 * C]
        ls_row = rows_sb[:, 3 * C : 4 * C]

        state = {"x1": None, "cp": 0}  # cp: copy-engine parity

        def chunk_done(k, acc):
            """acc[j, c] (PSUM) holds x1 for nodes k*128+j of this core."""
            gsz = min(GRP, KCH - (k // GRP) * GRP)
            k8 = k % GRP
            if k8 == 0:
                state["x1"] = x1pool.tile([P, GRP * C], f32, name="x1g", tag="x1g")
            x1 = state["x1"]
            # move PSUM -> SBUF, adding conv_bias
            nc.vector.tensor_tensor(
                x1[:, k8 * C : (k8 + 1) * C], acc[:], cb_row, op=Alu.add
            )
            if dbg:
                nc.sync.dma_start(
                    x1_d.ap()[:, k * C : (k + 1) * C],
                    x1[:, k8 * C : (k8 + 1) * C],
                )
            if k8 == gsz - 1:
                group_done(k // GRP, gsz, x1)

        def group_done(g, gsz, x1):
            v = x1[:].rearrange("p (k c) -> p k c", c=C)[:, :gsz, :]
            flat = x1[:, : gsz * C]
            s1 = tpool.tile([P, GRP], f32, tag="s1")
            s2 = tpool.tile([P, GRP], f32, tag="s2")
            sq = tpool.tile([P, GRP * C], f32, tag="sq")
            nc.vector.tensor_reduce(
                s1[:, :gsz], v, axis=mybir.AxisListType.X, op=Alu.add
            )
            nc.vector.tensor_tensor(sq[:, : gsz * C], flat, flat, op=Alu.mult)
            nc.vector.tensor_reduce(
                s2[:, :gsz],
                sq[:].rearrange("p (k c) -> p k c", c=C)[:, :gsz, :],
                axis=mybir.AxisListType.X,
                op=Alu.add,
            )
            nc.vector.tensor_scalar_mul(s1[:, :gsz], s1[:, :gsz], 1.0 / C)
            nc.vector.tensor_scalar_mul(s2[:, :gsz], s2[:, :gsz], 1.0 / C)
            mu2 = tpool.tile([P, GRP], f32, tag="mu2")
            nc.vector.tensor_tensor(mu2[:, :gsz], s1[:, :gsz], s1[:, :gsz], op=Alu.mult)
            var = tpool.tile([P, GRP], f32, tag="var")
            nc.vector.tensor_tensor(
                var[:, :gsz], s2[:, :gsz], mu2[:, :gsz], op=Alu.subtract
            )
            sd = tpool.tile([P, GRP], f32, tag="sd")
            nc.scalar.activation(
                sd[:, :gsz], var[:, :gsz], Act.Sqrt, bias=eps_sb[:, 0:1]
            )
            rs = tpool.tile([P, GRP], f32, tag="rs")
            nc.vector.reciprocal(rs[:, :gsz], sd[:, :gsz])

            xn = tpool.tile([P, GRP * C], f32, tag="xn")
            xnv = xn[:].rearrange("p (k c) -> p k c", c=C)[:, :gsz, :]
            nc.vector.tensor_tensor(
                xnv, v, s1[:, :gsz, None].to_broadcast([P, gsz, C]), op=Alu.subtract
            )
            nc.vector.tensor_tensor(
                xnv, xnv, rs[:, :gsz, None].to_broadcast([P, gsz, C]), op=Alu.mult
            )
            nc.vector.tensor_tensor(
                xnv, xnv, ga_row[:, None, :].to_broadcast([P, gsz, C]), op=Alu.mult
            )
            nc.vector.tensor_tensor(
                xnv, xnv, be_row[:, None, :].to_broadcast([P, gsz, C]), op=Alu.add
            )

            for k8 in range(gsz):
                k = g * GRP + k8
                nk = min(P, NS - k * P)
                # x1n^T for the MLP matmuls
                tp = epi_ps.tile([C, P], f32, tag="epi")
                nc.tensor.transpose(
                    tp[:], xn[:, k8 * C : (k8 + 1) * C], identity=ident_sb[:]
                )
                x1nT = tpool.tile([C, P], bf16, tag="x1nT")
                nc.scalar.activation(x1nT[:], tp[:], Act.Copy)
                hp = epi_ps.tile([P, H], f32, tag="epi")
                for j in range(H2):
                    nc.tensor.matmul(
                        hp[:, j * P : (j + 1) * P],
                        lhsT=w1t_sb[:, j * P : (j + 1) * P],
                        rhs=x1nT[:],
                        start=True,
                        stop=True,
                    )
                h1g = tpool.tile([P, H], bf16, tag="h1g")
                for j in range(H2):
                    nc.scalar.activation(
                        h1g[:, j * P : (j + 1) * P],
                        hp[:, j * P : (j + 1) * P],
                        Act.Gelu,
                        bias=b1t_sb[:, j : j + 1],
                    )
                h2p = epi_ps.tile([P, C], f32, tag="epi")
                for j in range(H2):
                    nc.tensor.matmul(
                        h2p[:],
                        lhsT=h1g[:, j * P : (j + 1) * P],
                        rhs=w2t_sb[:, j * C : (j + 1) * C],
                        start=(j == 0),
                        stop=(j == H2 - 1),
                    )
                xa = opool.tile([P, C], f32, tag="xa")
                nc.sync.dma_start(xa[:nk, :], xadj.ap()[k * P : k * P + nk, :])
                t1 = tpool.tile([P, C], f32, tag="t1")
                nc.vector.tensor_tensor(t1[:], h2p[:], ls_row, op=Alu.mult)
                yo = opool.tile([P, C], f32, tag="yo")
                nc.vector.tensor_tensor(
                    yo[:nk, :], t1[:nk, :], xa[:nk, :], op=Alu.add
                )
                nc.sync.dma_start(y.ap()[k * P : k * P + nk, :], yo[:nk, :])

        acc = None
        for b0 in range(0, Ttot, BT):
            tb = min(BT, Ttot - b0)
            idx_sl = gpool.tile([P, tb], i32, tag="idx")
            nc.sync.dma_start(idx_sl[:], sidx.ap()[:, b0 : b0 + tb])
            xg_sl = gpool.tile([P, tb * C], f32, tag="xg")
            nc.gpsimd.indirect_dma_start(
                out=xg_sl[:],
                out_offset=None,
                in_=xg.ap(),
                in_offset=bass.IndirectOffsetOnAxis(ap=idx_sl[:], axis=0),
            )
            kbt_sl = kpool.tile([KDIM, tb * P], bf16, tag="kbt")
            nc.sync.dma_start(kbt_sl[:], kbt.ap()[:, b0 * P : (b0 + tb) * P])

            kern_sl = mpool.tile([P, tb * C], bf16, tag="kern")
            for sub in range(0, tb, KSUB):
                sn = min(KSUB, tb - sub)
                kp = kmm_ps.tile([P, KSUB * C], f32, tag="kmm")
                for j in range(sn):
                    nc.tensor.matmul(
                        kp[:, j * C : (j + 1) * C],
                        lhsT=kbt_sl[:, (sub + j) * P : (sub + j + 1) * P],
                        rhs=wt_sb[:],
                        start=True,
                        stop=True,
                    )
                dstap = kern_sl[:, sub * C : (sub + sn) * C]
                if state["cp"] % 2 == 0:
                    nc.scalar.activation(dstap, kp[:, : sn * C], Act.Copy)
                else:
                    nc.vector.tensor_copy(dstap, kp[:, : sn * C])
                state["cp"] += 1

            msg_sl = mpool.tile([P, tb * C], bf16, tag="msg")
            nc.vector.tensor_tensor(msg_sl[:], xg_sl[:], kern_sl[:], op=Alu.mult)

            a_sl = apool.tile([P, tb * P], bf16, tag="oh")
            nc.vector.tensor_tensor(
                a_sl[:].rearrange("p (b j) -> p b j", j=P),
                doff_sb[:, b0 : b0 + tb, None].to_broadcast([P, tb, P]),
                iota_sb[:, None, :].to_broadcast([P, tb, P]),
                op=Alu.is_equal,
            )
            if dbg:
                nc.sync.dma_start(
                    xg_d.ap()[:, b0 * C : (b0 + tb) * C], xg_sl[:])
                nc.sync.dma_start(
                    kern_d.ap()[:, b0 * C : (b0 + tb) * C], kern_sl[:])
                nc.sync.dma_start(
                    msg_d.ap()[:, b0 * C : (b0 + tb) * C], msg_sl[:])
                nc.sync.dma_start(
                    a_d.ap()[:, b0 * P : (b0 + tb) * P], a_sl[:])

            for j in range(tb):
                t = b0 + j
                k = int(tile_chunk[t])
                first = t == int(tile_start[k])
                last = t == int(tile_start[k + 1]) - 1
                if first:
                    acc = acc_ps.tile([P, C], f32, tag="acc")
                nc.tensor.matmul(
                    acc[:],
                    lhsT=a_sl[:, j * P : (j + 1) * P],
                    rhs=msg_sl[:, j * C : (j + 1) * C],
                    start=first,
                    stop=last,
                )
                if last:
                    chunk_done(k, acc)

    nc.compile()
    return nc


def _prep_inputs(x, kernel_basis, edge_index, kernel_W, conv_bias, ln_gamma,
                 ln_beta, W1, b1, W2, b2, layer_scale, n_cores=NCORES):
    n_nodes, C = x.shape
    E, KDIM = kernel_basis.shape
    H = W1.shape[0]
    H2 = H // P
    pl = _plan(edge_index[0], edge_index[1], np.asarray(kernel_basis, np.float32),
               n_nodes, n_cores)
    NS, KCH, Ttot = pl["NS"], pl["KCH"], pl["Ttot"]

    xg = np.ascontiguousarray(np.asarray(x, np.float32))
    lsb2 = (np.asarray(layer_scale, np.float32) * np.asarray(b2, np.float32))[None, :]
    wt_h = np.ascontiguousarray(np.asarray(kernel_W, np.float32).T.astype(bfloat16))
    w1t_h = np.ascontiguousarray(np.asarray(W1, np.float32).T.astype(bfloat16))
    w2t_h = np.ascontiguousarray(
        np.asarray(W2, np.float32).T.reshape(H2, P, C).transpose(1, 0, 2)
        .reshape(P, H2 * C).astype(bfloat16))
    b1t_h = np.ascontiguousarray(np.asarray(b1, np.float32).reshape(H2, P).T)
    rows_h = np.tile(
        np.concatenate([np.asarray(conv_bias, np.float32),
                        np.asarray(ln_gamma, np.float32),
                        np.asarray(ln_beta, np.float32),
                        np.asarray(layer_scale, np.float32)])[None, :], (P, 1))
    iota_h = np.tile(np.arange(P, dtype=np.float32).astype(bfloat16)[None, :], (P, 1))

    in_maps = []
    for c in range(n_cores):
        kbt_h = np.ascontiguousarray(pl["kb_slots"][c].T.astype(bfloat16))
        sidx_h = np.ascontiguousarray(pl["src_slots"][c].reshape(Ttot, P).T)
        doff_h = np.ascontiguousarray(
            pl["off_slots"][c].reshape(Ttot, P).T.astype(bfloat16))
        xadj_h = np.ascontiguousarray(
            np.asarray(x[c * NS : (c + 1) * NS], np.float32) + lsb2)
        in_maps.append(dict(
            xg=xg, xadj=xadj_h, kbt=kbt_h, sidx=sidx_h, doff=doff_h,
            wt=wt_h, w1t=w1t_h, w2t=w2t_h, b1t=b1t_h, rows=rows_h, iota=iota_h,
        ))
    return pl, in_maps, dict(n_nodes=n_nodes, C=C, KDIM=KDIM, H=H)


def kernel(**inputs):
    x = np.asarray(inputs["x"], np.float32)
    edge_index = np.asarray(inputs["edge_index"])
    pl, in_maps, dims = _prep_inputs(
        x, inputs["kernel_basis"], edge_index, inputs["kernel_W"],
        inputs["conv_bias"], inputs["ln_gamma"], inputs["ln_beta"],
        inputs["W1"], inputs["b1"], inputs["W2"], inputs["b2"],
        inputs["layer_scale"])
    nc = _build_nc(dims["n_nodes"], dims["C"], dims["KDIM"], dims["H"],
                   pl["NS"], pl["KCH"], pl["Ttot"], pl["tile_start"],
                   pl["tile_chunk"])
    res = run_bass_kernel_spmd(nc, in_maps, core_ids=list(range(NCORES)))
    out = np.concatenate([res.results[c]["y"] for c in range(NCORES)], axis=0)
    return out.astype(np.float32)
